# revision 14
# baseline (speedup 1.0000x reference)
"""Trainium2 Bass kernel for the SE-gated Non-local block (rank-1 attention).

Math (per batch item b, x viewed as [C, N] with N = H*W):
    S[c]    = sum_n x[c, n]                      (spatial sum)
    hid     = relu((se_w1 / N) @ S + se_b1)      (SE bottleneck; 1/N folds the mean)
    gate    = sigmoid(se_w2 @ hid + se_b2)       [C]
    w3e     = gate * [g_w | theta_w | phi_w]     [C, 3]   (gate folded into projections)
    proj    = w3e.T @ x + [g_b, theta_b, phi_b]  [3, N]   (rows: g, theta, phi)
    s_raw   = sum_n proj[0] * proj[2]
    u       = proj[1] * s_raw                    [N]
    out     = x + A (outer) u + Bc               where
              inv = bn_gamma / sqrt(bn_var + eps)
              A   = W_w * inv / N                (1/N folds the f/N normalizer)
              Bc  = (W_b - bn_mean) * inv + bn_beta

Sharding: pure data parallel, 2 of the 16 batch items per core, params
replicated, no collectives.  Each batch item's x ([512, 4608] f32, 9.4 MB)
stays resident in SBUF between the stats pass and the output pass, so HBM
traffic is the minimum read-x + write-out (37.7 MB/core ~ 105 us at 358 GB/s).
"""

import numpy as np

B, C, H, W = 16, 512, 96, 48
N = H * W            # 4608
P = 128
KC = C // P          # 4 channel chunks
NB = 512             # free-dim block = one fp32 PSUM bank
NJ = N // NB         # 9
NCORES = 8
BPC = B // NCORES    # 2 batch items per core
SE_C = C // 16       # 32
BN_EPS = 1e-5

_CACHE = {}
LAST_RESULTS = None


def _build_bass(xpool_bufs=6, stage="full"):
    # stage: bisection aid — "loads" (copy only), "se", "proj", "rows", "full"
    S = {"loads": 0, "se": 1, "proj": 2, "rows": 3, "full": 4}[stage]
    import concourse.mybir as mybir
    from concourse.bacc import Bacc
    from concourse.tile import TileContext

    f32 = mybir.dt.float32
    AF = mybir.ActivationFunctionType
    ALU = mybir.AluOpType
    AX = mybir.AxisListType

    nc = Bacc()
    xs = nc.dram_tensor("xs", [BPC, C, N], f32, kind="ExternalInput")
    w1 = nc.dram_tensor("w1", [P, KC, SE_C], f32, kind="ExternalInput")
    w2 = nc.dram_tensor("w2", [SE_C, C], f32, kind="ExternalInput")
    b1 = nc.dram_tensor("b1", [SE_C, 1], f32, kind="ExternalInput")
    b2 = nc.dram_tensor("b2", [P, KC], f32, kind="ExternalInput")
    w3 = nc.dram_tensor("w3", [P, KC, 3], f32, kind="ExternalInput")
    pb = nc.dram_tensor("pb", [3, 1], f32, kind="ExternalInput")
    ab = nc.dram_tensor("ab", [2, C], f32, kind="ExternalInput")   # rows: A, Bc
    sel = nc.dram_tensor("sel", [3, 2], f32, kind="ExternalInput")  # e1|e2
    uo = nc.dram_tensor("uo", [2, N], f32, kind="ExternalInput")   # [junk; ones]
    out_d = nc.dram_tensor("out", [BPC, C, N], f32, kind="ExternalOutput")

    with TileContext(nc) as tc:
        with (
            tc.tile_pool(name="wpool", bufs=1) as wpool,
            tc.tile_pool(name="xpool", bufs=xpool_bufs) as xpool,
            tc.tile_pool(name="ppool", bufs=1) as ppool,
            tc.tile_pool(name="spool", bufs=2) as spool,
            tc.tile_pool(name="ps_se", bufs=2, space="PSUM") as ps_se,
            tc.tile_pool(name="ps_pp", bufs=2, space="PSUM") as ps_pp,
            tc.tile_pool(name="ps_po", bufs=4, space="PSUM") as ps_po,
        ):
            w1t = wpool.tile([P, KC, SE_C], f32, tag="w1t")
            w2t = wpool.tile([SE_C, C], f32, tag="w2t")
            b1t = wpool.tile([SE_C, 1], f32, tag="b1t")
            b2t = wpool.tile([P, KC], f32, tag="b2t")
            w3t = wpool.tile([P, KC, 3], f32, tag="w3t")
            pbt = wpool.tile([3, 1], f32, tag="pbt")
            abt = wpool.tile([2, C], f32, tag="abt")
            selt = wpool.tile([3, 2], f32, tag="selt")
            # phi row staged on partition 0 (compute ops cannot address
            # partitions 1/2 directly: SBUF partition offsets must be 32-aligned)
            pht = wpool.tile([1, N], f32, tag="pht")
            # u2: row 0 = u (rewritten per batch), row 1 = ones (from HBM)
            u2 = wpool.tile([2, N], f32, tag="u2")

            for t, d in ((w1t, w1), (w2t, w2), (b1t, b1), (b2t, b2),
                         (w3t, w3), (pbt, pb), (abt, ab), (selt, sel),
                         (u2, uo)):
                nc.gpsimd.dma_start(out=t[:], in_=d[:])

            for b in range(BPC):
                # ---- load x (resident for the whole batch item) ----
                xts = []
                for k in range(KC):
                    xt = xpool.tile([P, N], f32, tag="xt")
                    nc.sync.dma_start(out=xt[:], in_=xs[b, k * P:(k + 1) * P, :])
                    xts.append(xt)

                # ---- spatial sums -> SE gate ----
                xp = spool.tile([P, KC], f32, tag="xp")
                for k in range(KC):
                    nc.vector.reduce_sum(out=xp[:, k:k + 1], in_=xts[k][:], axis=AX.X)

                if S >= 1:
                    php = ps_se.tile([SE_C, 1], f32, tag="ps_se")
                    for k in range(KC):
                        nc.tensor.matmul(php[:], w1t[:, k, :], xp[:, k:k + 1],
                                         start=(k == 0), stop=(k == KC - 1))
                    hid = spool.tile([SE_C, 1], f32, tag="hid")
                    nc.scalar.activation(out=hid[:], in_=php[:], func=AF.Relu,
                                         bias=b1t[:], scale=1.0)

                    gate = spool.tile([P, KC], f32, tag="gate")
                    for k in range(KC):
                        gp = ps_se.tile([P, 1], f32, tag="ps_se")
                        nc.tensor.matmul(gp[:], w2t[:, k * P:(k + 1) * P], hid[:],
                                         start=True, stop=True)
                        nc.scalar.activation(out=gate[:, k:k + 1], in_=gp[:],
                                             func=AF.Sigmoid, bias=b2t[:, k:k + 1],
                                             scale=1.0)

                if S >= 2:
                    # ---- gated projection weights, then proj = w3e.T @ x ----
                    w3e = spool.tile([P, KC, 3], f32, tag="w3e")
                    for k in range(KC):
                        nc.vector.tensor_scalar_mul(out=w3e[:, k, :],
                                                    in0=w3t[:, k, :],
                                                    scalar1=gate[:, k:k + 1])

                    proj = ppool.tile([3, N], f32, tag="proj")
                    for j in range(NJ):
                        pp = ps_pp.tile([3, NB], f32, tag="pp")
                        for k in range(KC):
                            nc.tensor.matmul(pp[:], w3e[:, k, :],
                                             xts[k][:, j * NB:(j + 1) * NB],
                                             start=(k == 0), stop=(k == KC - 1))
                        nc.vector.tensor_scalar_add(
                            out=proj[:, j * NB:(j + 1) * NB],
                            in0=pp[:], scalar1=pbt[:])

                if S >= 3:
                    # ---- stage theta -> u2 row 0, phi -> pht via one-hot
                    # matmuls (PE can read partitions 1/2; DVE cannot) ----
                    for j in range(NJ):
                        pe_r = ps_pp.tile([1, NB], f32, tag="pp")
                        nc.tensor.matmul(pe_r[:], selt[:, 0:1],
                                         proj[:, j * NB:(j + 1) * NB],
                                         start=True, stop=True)
                        nc.vector.tensor_copy(out=u2[0:1, j * NB:(j + 1) * NB],
                                              in_=pe_r[:])
                        pe_r2 = ps_pp.tile([1, NB], f32, tag="pp")
                        nc.tensor.matmul(pe_r2[:], selt[:, 1:2],
                                         proj[:, j * NB:(j + 1) * NB],
                                         start=True, stop=True)
                        nc.vector.tensor_copy(out=pht[:, j * NB:(j + 1) * NB],
                                              in_=pe_r2[:])
                    # ---- s_raw = <g, phi> ; u = theta * s_raw ----
                    srw = spool.tile([1, 1], f32, tag="srw")
                    nc.vector.tensor_mul(out=proj[0:1, :], in0=proj[0:1, :],
                                         in1=pht[:])
                    nc.vector.reduce_sum(out=srw[:], in_=proj[0:1, :], axis=AX.X)
                    nc.vector.tensor_scalar_mul(out=u2[0:1, :], in0=u2[0:1, :],
                                                scalar1=srw[:])

                # ---- out = x + A (outer) u + Bc (outer) ones, store ----
                for k in range(KC):
                    if S >= 4:
                        for j in range(NJ):
                            po = ps_po.tile([P, NB], f32, tag="po")
                            nc.tensor.matmul(po[:], abt[:, k * P:(k + 1) * P],
                                             u2[:, j * NB:(j + 1) * NB],
                                             start=True, stop=True)
                            nc.vector.tensor_add(
                                out=xts[k][:, j * NB:(j + 1) * NB],
                                in0=xts[k][:, j * NB:(j + 1) * NB], in1=po[:])
                    nc.scalar.dma_start(out=out_d[b, k * P:(k + 1) * P, :],
                                        in_=xts[k][:])

    nc.finalize()  # runs Bacc compile passes (wait splitting, reg alloc, ...)
    return nc


def kernel(**inputs):
    global LAST_RESULTS
    from concourse.bass_utils import run_bass_kernel_spmd

    a = {k: np.asarray(v, dtype=np.float32) for k, v in inputs.items()}
    x = np.ascontiguousarray(a["x"]).reshape(B, C, N)

    inv = a["bn_gamma"] / np.sqrt(a["bn_var"] + BN_EPS)
    A = (a["W_w"] * inv / N).astype(np.float32)
    Bc = ((a["W_b"] - a["bn_mean"]) * inv + a["bn_beta"]).astype(np.float32)

    w1h = np.ascontiguousarray(
        (a["se_w1"] / N).T.reshape(KC, P, SE_C).transpose(1, 0, 2)).astype(np.float32)
    w2h = np.ascontiguousarray(a["se_w2"].T).astype(np.float32)
    b1h = np.ascontiguousarray(a["se_b1"].reshape(SE_C, 1))
    b2h = np.ascontiguousarray(a["se_b2"].reshape(KC, P).T)
    w3h = np.ascontiguousarray(
        np.stack([a["g_w"], a["theta_w"], a["phi_w"]], axis=1)
        .reshape(KC, P, 3).transpose(1, 0, 2)).astype(np.float32)
    pbh = np.array([[a["g_b"]], [a["theta_b"]], [a["phi_b"]]], dtype=np.float32)
    abh = np.ascontiguousarray(np.stack([A, Bc]))              # (2, C)
    selh = np.array([[0, 0], [1, 0], [0, 1]], dtype=np.float32)  # e1 | e2
    uoh = np.zeros((2, N), dtype=np.float32)
    uoh[1, :] = 1.0

    if "nc" not in _CACHE:
        _CACHE["nc"] = _build_bass()
    nc = _CACHE["nc"]

    in_maps = []
    for c in range(NCORES):
        in_maps.append({
            "xs": np.ascontiguousarray(x[c * BPC:(c + 1) * BPC]),
            "w1": w1h, "w2": w2h, "b1": b1h, "b2": b2h,
            "w3": w3h, "pb": pbh, "ab": abh, "sel": selh, "uo": uoh,
        })

    res = run_bass_kernel_spmd(nc, in_maps, core_ids=list(range(NCORES)))
    LAST_RESULTS = res

    out = np.concatenate([res.results[c]["out"] for c in range(NCORES)], axis=0)
    return np.ascontiguousarray(out.reshape(B, C, H, W))


# revision 16
# speedup vs baseline: 1.5970x; 1.5970x over previous
"""Trainium2 Bass kernel for the SE-gated Non-local block (rank-1 attention).

Math (per batch item b, x viewed as [C, N] with N = H*W):
    S[c]    = sum_n x[c, n]                      (spatial sum)
    hid     = relu((se_w1 / N) @ S + se_b1)      (SE bottleneck; 1/N folds the mean)
    gate    = sigmoid(se_w2 @ hid + se_b2)       [C]
    w3e     = gate * [g_w | theta_w | phi_w]     [C, 3]   (gate folded into projections)
    proj    = w3e.T @ x + [g_b, theta_b, phi_b]  [3, N]   (rows: g, theta, phi)
    s_raw   = sum_n proj[0] * proj[2]
    out     = x + (A * s_raw) (outer) theta + Bc (outer) ones   where
              inv = bn_gamma / sqrt(bn_var + eps)
              A   = W_w * inv / N                (1/N folds the f/N normalizer)
              Bc  = (W_b - bn_mean) * inv + bn_beta

Precision split: the correction term A*s*theta has rms ~5e-6 vs |x| ~ 1, so the
whole gate/projection path runs in bf16 (costs ~1e-8 output rel err); x and the
Bc affine stay exact f32.

Sharding: pure data parallel, 2 of the 16 batch items per core, params
replicated, no collectives.  Each batch item's x ([512, 4608] f32, 9.4 MB)
stays resident in SBUF between the stats pass and the output pass, so HBM
traffic is near the minimum read-x + write-out (37.7 MB/core ~ 105 us).

Engine budget per core: DMA ~105 us (bound), ACT ~70 us (bf16 cast+rowsum,
affine), DVE ~45 us (psum copies, final adds), PE ~45 us (bf16 proj matmuls).
"""

import numpy as np

B, C, H, W = 16, 512, 96, 48
N = H * W            # 4608
P = 128
KC = C // P          # 4 channel chunks
NB = 512             # free-dim block = one fp32 PSUM bank
NJ = N // NB         # 9
NCORES = 8
BPC = B // NCORES    # 2 batch items per core
SE_C = C // 16       # 32
BN_EPS = 1e-5

_CACHE = {}
LAST_RESULTS = None


def _build_bass(xpool_bufs=4, stage="full"):
    # stage: bisection aid — "loads", "se", "proj", "rows", "full"
    S = {"loads": 0, "se": 1, "proj": 2, "rows": 3, "full": 4}[stage]
    import concourse.mybir as mybir
    from concourse.bacc import Bacc
    from concourse.tile import TileContext

    f32 = mybir.dt.float32
    bf16 = mybir.dt.bfloat16
    AF = mybir.ActivationFunctionType
    AX = mybir.AxisListType

    nc = Bacc()
    xs = nc.dram_tensor("xs", [BPC, C, N], f32, kind="ExternalInput")
    w1 = nc.dram_tensor("w1", [P, KC, SE_C], f32, kind="ExternalInput")
    w2 = nc.dram_tensor("w2", [SE_C, C], f32, kind="ExternalInput")
    b1 = nc.dram_tensor("b1", [SE_C, 1], f32, kind="ExternalInput")
    b2 = nc.dram_tensor("b2", [P, KC], f32, kind="ExternalInput")
    w3 = nc.dram_tensor("w3", [P, KC, 3], bf16, kind="ExternalInput")
    pb = nc.dram_tensor("pb", [3, 1], f32, kind="ExternalInput")
    at = nc.dram_tensor("at", [P, KC], f32, kind="ExternalInput")   # A chunks
    bc = nc.dram_tensor("bc", [P, KC], f32, kind="ExternalInput")   # Bc chunks
    on1 = nc.dram_tensor("on1", [1, P], f32, kind="ExternalInput")  # ones row
    out_d = nc.dram_tensor("out", [BPC, C, N], f32, kind="ExternalOutput")
    # per-batch DRAM scratch for the theta/phi rows (partition 1/2 of proj are
    # unreachable by compute engines; bounce through HBM to partition 0 / bcast)
    tp_scr = nc.dram_tensor("tp_scr", [BPC, 2, N], bf16)

    with TileContext(nc) as tc:
        with (
            tc.tile_pool(name="wpool", bufs=1) as wpool,
            tc.tile_pool(name="xpool", bufs=xpool_bufs) as xpool,
            tc.tile_pool(name="bpool", bufs=4) as bpool,
            tc.tile_pool(name="ppool", bufs=1) as ppool,
            tc.tile_pool(name="spool", bufs=2) as spool,
            tc.tile_pool(name="tpool", bufs=2) as tpool,
            tc.tile_pool(name="ps_se", bufs=2, space="PSUM") as ps_se,
            tc.tile_pool(name="ps_pp", bufs=3, space="PSUM") as ps_pp,
        ):
            w1t = wpool.tile([P, KC, SE_C], f32, tag="w1t")
            w2t = wpool.tile([SE_C, C], f32, tag="w2t")
            b1t = wpool.tile([SE_C, 1], f32, tag="b1t")
            b2t = wpool.tile([P, KC], f32, tag="b2t")
            w3t = wpool.tile([P, KC, 3], bf16, tag="w3t")
            pbt = wpool.tile([3, 1], f32, tag="pbt")
            att = wpool.tile([P, KC], f32, tag="att")
            bct = wpool.tile([P, KC], f32, tag="bct")
            ont = wpool.tile([1, P], f32, tag="ont")
            pht = wpool.tile([1, N], bf16, tag="pht")     # phi row (partition 0)
            ubt = wpool.tile([P, N], bf16, tag="ubt")     # theta bcast to 128p

            for t, d in ((w1t, w1), (w2t, w2), (b1t, b1), (b2t, b2),
                         (w3t, w3), (pbt, pb), (att, at), (bct, bc),
                         (ont, on1)):
                nc.gpsimd.dma_start(out=t[:], in_=d[:])

            for b in range(BPC):
                # ---- load x (f32, resident for the whole batch item) ----
                xts = []
                for k in range(KC):
                    xt = xpool.tile([P, N], f32, tag="xt")
                    nc.sync.dma_start(out=xt[:], in_=xs[b, k * P:(k + 1) * P, :])
                    xts.append(xt)

                # ---- bf16 working copy of x + spatial sums (one ACT op) ----
                xbs = []
                xp = spool.tile([P, KC], f32, tag="xp")
                for k in range(KC):
                    xb = bpool.tile([P, N], bf16, tag="xb")
                    nc.scalar.activation(out=xb[:], in_=xts[k][:],
                                         func=AF.Identity,
                                         accum_out=xp[:, k:k + 1])
                    xbs.append(xb)

                if S >= 1:
                    # ---- SE gate ----
                    php = ps_se.tile([SE_C, 1], f32, tag="ps_se")
                    for k in range(KC):
                        nc.tensor.matmul(php[:], w1t[:, k, :], xp[:, k:k + 1],
                                         start=(k == 0), stop=(k == KC - 1))
                    hid = spool.tile([SE_C, 1], f32, tag="hid")
                    nc.scalar.activation(out=hid[:], in_=php[:], func=AF.Relu,
                                         bias=b1t[:], scale=1.0)

                    gate = spool.tile([P, KC], f32, tag="gate")
                    for k in range(KC):
                        gp = ps_se.tile([P, 1], f32, tag="ps_se")
                        nc.tensor.matmul(gp[:], w2t[:, k * P:(k + 1) * P], hid[:],
                                         start=True, stop=True)
                        nc.scalar.activation(out=gate[:, k:k + 1], in_=gp[:],
                                             func=AF.Sigmoid, bias=b2t[:, k:k + 1],
                                             scale=1.0)

                if S >= 2:
                    # ---- gated projections: proj = w3e.T @ x (bf16 PE) ----
                    w3e = spool.tile([P, KC, 3], bf16, tag="w3e")
                    for k in range(KC):
                        nc.vector.tensor_scalar_mul(out=w3e[:, k, :],
                                                    in0=w3t[:, k, :],
                                                    scalar1=gate[:, k:k + 1])

                    proj = ppool.tile([3, N], bf16, tag="proj")
                    for j in range(NJ):
                        pp = ps_pp.tile([3, NB], f32, tag="pp")
                        for k in range(KC):
                            nc.tensor.matmul(pp[:], w3e[:, k, :],
                                             xbs[k][:, j * NB:(j + 1) * NB],
                                             start=(k == 0), stop=(k == KC - 1))
                        nc.vector.tensor_scalar_add(
                            out=proj[:, j * NB:(j + 1) * NB],
                            in0=pp[:], scalar1=pbt[:])

                if S >= 3:
                    # ---- theta/phi rows -> HBM; phi back to partition 0,
                    #      theta back broadcast to all 128 partitions ----
                    nc.gpsimd.dma_start(out=tp_scr[b], in_=proj[1:3, :])
                    nc.sync.dma_start(out=pht[:], in_=tp_scr[b, 1:2, :])
                    nc.sync.dma_start(
                        out=ubt[:],
                        in_=tp_scr[b, 0:1, :].partition_broadcast(P)[:, 0, :])
                    # ---- s_raw = <g, phi> (in place over phi row) ----
                    srw = spool.tile([1, 1], f32, tag="srw")
                    nc.vector.tensor_mul(out=pht[:], in0=proj[0:1, :], in1=pht[:])
                    nc.vector.reduce_sum(out=srw[:], in_=pht[:], axis=AX.X)
                    # ---- broadcast s_raw to 128 partitions via tiny matmul,
                    #      scale A by it: as_t = A * s_raw ----
                    sb = ps_se.tile([P, 1], f32, tag="ps_se")
                    nc.tensor.matmul(sb[:], ont[:], srw[:], start=True, stop=True)
                    ast = spool.tile([P, KC], f32, tag="ast")
                    nc.vector.tensor_scalar_mul(out=ast[:], in0=att[:],
                                                scalar1=sb[:])

                # ---- out = x + (A*s) * theta_bcast + Bc, store ----
                for k in range(KC):
                    if S >= 4:
                        t1 = tpool.tile([P, N], f32, tag="t1")
                        nc.scalar.activation(out=t1[:], in_=ubt[:],
                                             func=AF.Identity,
                                             scale=ast[:, k:k + 1],
                                             bias=bct[:, k:k + 1])
                        nc.vector.tensor_add(out=xts[k][:], in0=xts[k][:],
                                             in1=t1[:])
                    nc.scalar.dma_start(out=out_d[b, k * P:(k + 1) * P, :],
                                        in_=xts[k][:])

    nc.finalize()  # runs Bacc compile passes (wait splitting, reg alloc, ...)
    return nc


def kernel(**inputs):
    global LAST_RESULTS
    from concourse.bass_utils import run_bass_kernel_spmd

    a = {k: np.asarray(v, dtype=np.float32) for k, v in inputs.items()}
    x = np.ascontiguousarray(a["x"]).reshape(B, C, N)

    inv = a["bn_gamma"] / np.sqrt(a["bn_var"] + BN_EPS)
    A = (a["W_w"] * inv / N).astype(np.float32)
    Bc = ((a["W_b"] - a["bn_mean"]) * inv + a["bn_beta"]).astype(np.float32)

    w1h = np.ascontiguousarray(
        (a["se_w1"] / N).T.reshape(KC, P, SE_C).transpose(1, 0, 2)).astype(np.float32)
    w2h = np.ascontiguousarray(a["se_w2"].T).astype(np.float32)
    b1h = np.ascontiguousarray(a["se_b1"].reshape(SE_C, 1))
    b2h = np.ascontiguousarray(a["se_b2"].reshape(KC, P).T)
    import ml_dtypes
    w3h = np.ascontiguousarray(
        np.stack([a["g_w"], a["theta_w"], a["phi_w"]], axis=1)
        .reshape(KC, P, 3).transpose(1, 0, 2)).astype(ml_dtypes.bfloat16)
    pbh = np.array([[a["g_b"]], [a["theta_b"]], [a["phi_b"]]], dtype=np.float32)
    ath = np.ascontiguousarray(A.reshape(KC, P).T)
    bch = np.ascontiguousarray(Bc.reshape(KC, P).T)
    onh = np.ones((1, P), dtype=np.float32)

    if "nc" not in _CACHE:
        _CACHE["nc"] = _build_bass()
    nc = _CACHE["nc"]

    in_maps = []
    for c in range(NCORES):
        in_maps.append({
            "xs": np.ascontiguousarray(x[c * BPC:(c + 1) * BPC]),
            "w1": w1h, "w2": w2h, "b1": b1h, "b2": b2h,
            "w3": w3h, "pb": pbh, "at": ath, "bc": bch, "on1": onh,
        })

    res = run_bass_kernel_spmd(nc, in_maps, core_ids=list(range(NCORES)))
    LAST_RESULTS = res

    out = np.concatenate([res.results[c]["out"] for c in range(NCORES)], axis=0)
    return np.ascontiguousarray(out.reshape(B, C, H, W))
